# revision 28
# baseline (speedup 1.0000x reference)
"""Trainium2 Bass kernel for a 3-layer dense-adjacency GCN decoder.

Problem (per batch graph): 3x GCN layer (msg = h@W + b; agg = A @ msg; relu)
followed by output projection + node mask. B=8 graphs of N=2048 nodes,
latent=64, hidden=128, out=64. Batch-parallel: one graph per NeuronCore.

Per-core plan (all-bf16 datapath, everything 1 PE-cycle/row):
  - The host hands each core A^T in bf16, panel-major: panel i holds the
    512 target columns i*512..(i+1)*512 for all 2048 source rows.  Each
    panel streams HBM->SBUF as 4 quarter DMAs (fully contiguous rows);
    A^T stays SBUF-resident and is reused by all 3 layers.  bf16 halves
    the HBM traffic of the dominant read (16.8MB -> 8.4MB per core).
  - Features are kept feature-major (h^T: [d, n] bf16; X^T comes
    pre-transposed+cast from the host).  msg is produced NODE-MAJOR
    directly -- msg chunk c = (h^T[:, c*128:+128] stationary) @ W -- so
    no PE transposes anywhere in the pipeline; one PSUM round trip per
    value.
  - Aggregation chunk i accumulates (msg block j stationary) @ (A^T
    panel-i slice, 512-wide moving) over the 16 j-blocks; ReLU rides the
    PSUM->SBUF evacuation (alternating ACT/DVE) straight into the next
    h^T.
  - Schedule: msg0 runs under the stream head; layer-0 agg chunk i is
    paced by panel i's quarter DMAs; msg1 + layer-1 partial aggregation
    steps fill the PE slack under the stream (emitted BEFORE each
    quarter's agg0 group so the in-order PE never head-of-line blocks on
    the DMA).  After the stream the tail is a software pipeline:
    agg1 tail -> relu1 -> msg2 (interleaved) -> agg2 -> relu2 -> proj
    (delay-by-one) -> masked evac -> Y DMA halves.
  - Bias / node-mask are supported via build flags resolved on the host
    at call time (bias = one extra rank-1 accumulating matmul per msg
    chunk; mask = per-partition ACT scale on the output evacuation).
    With zero biases / unit mask those instructions are not emitted.
"""

import functools

import numpy as np

import concourse.bass as bass
import concourse.bacc as bacc
import concourse.tile as tile
from concourse import mybir
from concourse.bass_utils import run_bass_kernel_spmd

B = 8
N = 2048
NT = N // 128  # 16 node blocks
NP = 4         # target panels (512 columns each)
NQ = 4         # quarter DMAs per panel
LAT = 64
HID = 128
ODIM = 64
N_CORES = 8

PANW = NT * 512  # at_t columns per panel
QW = PANW // NQ  # at_t columns per quarter DMA

F32 = mybir.dt.float32
BF16 = mybir.dt.bfloat16
Act = mybir.ActivationFunctionType


@functools.lru_cache(maxsize=4)
def _build(has_bias: bool, has_mask: bool):
    nc = bacc.Bacc(None, target_bir_lowering=False, debug=False)

    ATP_d = nc.declare_dram_parameter("ATP", [NP, 128, PANW], BF16,
                                      isOutput=False)
    XT_d = nc.declare_dram_parameter("XT", [LAT, N], BF16, isOutput=False)
    W0_d = nc.declare_dram_parameter("W0", [LAT, HID], BF16, isOutput=False)
    W1_d = nc.declare_dram_parameter("W1", [HID, HID], BF16, isOutput=False)
    W2_d = nc.declare_dram_parameter("W2", [HID, HID], BF16, isOutput=False)
    WO_d = nc.declare_dram_parameter("WO", [HID, ODIM], BF16, isOutput=False)
    if has_bias:
        B0_d = nc.declare_dram_parameter("B0", [1, HID], BF16, isOutput=False)
        B1_d = nc.declare_dram_parameter("B1", [1, HID], BF16, isOutput=False)
        B2_d = nc.declare_dram_parameter("B2", [1, HID], BF16, isOutput=False)
        BO_d = nc.declare_dram_parameter("BO", [1, ODIM], BF16, isOutput=False)
        ONE_d = nc.declare_dram_parameter("ONE", [1, 128], BF16,
                                          isOutput=False)
    if has_mask:
        MSK_d = nc.declare_dram_parameter("MSK", [128, NT], F32,
                                          isOutput=False)
    Y_d = nc.declare_dram_parameter("Y", [N, ODIM], F32, isOutput=True)
    Y3 = Y_d[:].rearrange("(t p) f -> p t f", p=128)

    with tile.TileContext(nc) as tc:
        with (
            tc.tile_pool(name="const", bufs=1) as constp,
            tc.tile_pool(name="at", bufs=1) as atp,
            tc.tile_pool(name="x", bufs=1) as xp,
            tc.tile_pool(name="ht", bufs=3) as htp,
            tc.tile_pool(name="msg", bufs=3) as msgp,
            tc.tile_pool(name="out", bufs=1) as outp,
            tc.tile_pool(name="agg", bufs=5, space=bass.MemorySpace.PSUM)
            as aggp,
            tc.tile_pool(name="mp", bufs=3, space=bass.MemorySpace.PSUM)
            as mpp,
        ):
            at_t = atp.tile([128, NP * PANW], BF16, tag="at")

            # ---- sync queue: W0 + X^T (halved) ride ahead of the A^T
            # stream (the wire serves this queue in order; anything on the
            # other queue starves behind the 8.4MB stream) ----
            w0 = constp.tile([LAT, HID], BF16, tag="w0")
            nc.sync.dma_start(w0[:], W0_d[:])
            hT0 = xp.tile([LAT, N], BF16, tag="x")
            nc.sync.dma_start(hT0[:], XT_d[:])
            for i in range(NP):
                # panel 0 streams in eighths so agg0 starts on the first
                # piece; later panels use quarters (fewer issue slots)
                npieces = 8 if i == 0 else NQ
                pw = PANW // npieces
                for q in range(npieces):
                    c0 = i * PANW + q * pw
                    nc.sync.dma_start(
                        at_t[:, c0 : c0 + pw],
                        ATP_d[i, :, q * pw : (q + 1) * pw],
                    )

            # ---- gpsimd queue: the later constants ----
            w1 = constp.tile([HID, HID], BF16, tag="w1")
            nc.gpsimd.dma_start(w1[:], W1_d[:])
            w2 = constp.tile([HID, HID], BF16, tag="w2")
            nc.gpsimd.dma_start(w2[:], W2_d[:])
            wo = constp.tile([HID, ODIM], BF16, tag="wo")
            nc.gpsimd.dma_start(wo[:], WO_d[:])
            if has_bias:
                one_t = constp.tile([1, 128], BF16, tag="one")
                nc.gpsimd.dma_start(one_t[:], ONE_d[:])
                bcols = []
                for name, d_, bd in (("b0", HID, B0_d), ("b1", HID, B1_d),
                                     ("b2", HID, B2_d), ("bo", ODIM, BO_d)):
                    bt = constp.tile([1, d_], BF16, tag=name)
                    nc.gpsimd.dma_start(bt[:], bd[:])
                    bcols.append(bt)
            if has_mask:
                mskn = constp.tile([128, NT], F32, tag="mskn")
                nc.gpsimd.dma_start(mskn[:], MSK_d[:])

            hts = [hT0]
            for l in range(3):
                hts.append(htp.tile([HID, N], BF16, tag="ht",
                                    name=f"hT{l + 1}"))
            msgs = [msgp.tile([128, N], BF16, tag="msg", name=f"msg{l}")
                    for l in range(3)]
            out_sb = outp.tile([128, NT * ODIM], F32, tag="out")

            ws = [w0, w1, w2]
            din = [LAT, HID, HID]

            tog = [0]

            def alt():
                tog[0] ^= 1
                return tog[0]

            # ---- msg stage: node-major msg chunks, 4 per PSUM bank ----
            mp_tiles = {}

            def emit_msg_mm(l, c):
                g, q = divmod(c, 4)
                if q == 0:
                    mp_tiles[(l, g)] = mpp.tile([128, 512], F32, tag="mp",
                                                name=f"mp{l}_{g}")
                mp = mp_tiles[(l, g)]
                dst = mp[:, q * HID : (q + 1) * HID]
                nc.tensor.matmul(
                    dst,
                    hts[l][0 : din[l], c * 128 : (c + 1) * 128],
                    ws[l][:],
                    start=True,
                    stop=not has_bias,
                )
                if has_bias:
                    nc.tensor.matmul(
                        dst, one_t[:], bcols[l][:], start=False, stop=True
                    )

            def emit_msg_evac(l, g):
                # halves on both engines concurrently: dependents unblock
                # in ~half the evacuation latency
                mp = mp_tiles.pop((l, g))
                dst = msgs[l][:, g * 512 : (g + 1) * 512]
                nc.scalar.activation(dst[:, 0:256], mp[:, 0:256], Act.Identity)
                nc.vector.tensor_copy(dst[:, 256:512], mp[:, 256:512])

            # ---- aggregation: 16 accumulating j-steps per 512-col chunk ----
            agg_tiles = {}
            agg_cnt = {}

            def emit_agg_mm(l, i, j):
                key = (l, i)
                if key not in agg_tiles:
                    agg_tiles[key] = aggp.tile([128, 512], F32, tag="agg",
                                               name=f"agg{l}_{i}")
                    agg_cnt[key] = 0
                n = agg_cnt[key]
                nc.tensor.matmul(
                    agg_tiles[key][:],
                    msgs[l][:, j * 128 : (j + 1) * 128],
                    at_t[:, i * PANW + j * 512 : i * PANW + (j + 1) * 512],
                    start=(n == 0),
                    stop=(n == NT - 1),
                )
                agg_cnt[key] = n + 1

            def emit_relu(l, i):
                ap_ps = agg_tiles.pop((l, i))
                dst = hts[l + 1][:, i * 512 : (i + 1) * 512]
                nc.scalar.activation(dst[:, 0:256], ap_ps[:, 0:256], Act.Relu)
                nc.vector.tensor_scalar_max(
                    dst[:, 256:512], ap_ps[:, 256:512], 0.0
                )

            # ---- projection: node-major out chunks, 8 per PSUM bank ----
            pp_tiles = {}

            def emit_proj_mm(c):
                b, s = divmod(c, 8)
                if s == 0:
                    pp_tiles[b] = mpp.tile([128, 512], F32, tag="mp",
                                           name=f"pp{b}")
                dst = pp_tiles[b][:, s * ODIM : (s + 1) * ODIM]
                nc.tensor.matmul(
                    dst,
                    hts[3][:, c * 128 : (c + 1) * 128],
                    wo[:],
                    start=True,
                    stop=not has_bias,
                )
                if has_bias:
                    nc.tensor.matmul(
                        dst, one_t[:], bcols[3][:], start=False, stop=True
                    )

            def emit_proj_out(bk, half):
                # evacuate + DMA one half-bank (4 node blocks, 128KB)
                pp = pp_tiles[bk] if half == 0 else pp_tiles.pop(bk)
                if has_mask:
                    for s in range(4 * half, 4 * half + 4):
                        c = bk * 8 + s
                        col = mskn[:, c : c + 1]
                        dst = out_sb[:, c * ODIM : (c + 1) * ODIM]
                        src = pp[:, s * ODIM : (s + 1) * ODIM]
                        if alt():
                            nc.scalar.activation(dst, src, Act.Copy,
                                                 scale=col)
                        else:
                            nc.vector.tensor_scalar_mul(dst, src, col)
                else:
                    o = bk * 512 + half * 256
                    dst = out_sb[:, o : o + 256]
                    src = pp[:, half * 256 : half * 256 + 256]
                    nc.scalar.activation(dst[:, 0:128], src[:, 0:128],
                                         Act.Identity)
                    nc.vector.tensor_copy(dst[:, 128:256], src[:, 128:256])
                t0 = bk * 8 + half * 4
                nc.sync.dma_start(
                    Y3[:, t0 : t0 + 4, :],
                    out_sb[:].rearrange("p (t f) -> p t f", f=ODIM)[
                        :, t0 : t0 + 4, :
                    ],
                )

            # ================= schedule =================
            # msg0 for all 16 chunks (depends only on X^T + W0)
            for c in range(NT):
                emit_msg_mm(0, c)
                if c % 4 == 3:
                    emit_msg_evac(0, c // 4)

            # layer-0 agg paced by the stream; msg1 + agg1 partials fill
            # the slack.  Ready work is emitted BEFORE each quarter's
            # agg0 group so the in-order PE never blocks on the DMA.
            agg1_next_j = [0, 0, 0, 0]

            def agg1_avail(jmax, ipmax):
                steps = []
                for ip in range(ipmax + 1):
                    while agg1_next_j[ip] < jmax:
                        steps.append((ip, agg1_next_j[ip]))
                        agg1_next_j[ip] += 1
                return steps

            for i in range(NP):
                for q in range(NQ):
                    if i >= 1 and q == 0:
                        # all msg1 work for the finished panel up front --
                        # the LS stall lands inside the panel-boundary
                        # DMA wait
                        for c in range(4 * (i - 1), 4 * i):
                            emit_msg_mm(1, c)
                        emit_msg_evac(1, i - 1)
                    if i >= 1 and q >= 1:
                        # agg1 partials fill the slack under the stream
                        for ip, jj in agg1_avail(4 * i, i - 1):
                            emit_agg_mm(1, ip, jj)
                    for j in range(4 * q, 4 * q + 4):
                        emit_agg_mm(0, i, j)
                emit_relu(0, i)

            # stream done: msg1 tail
            for c in range(12, 16):
                emit_msg_mm(1, c)
            emit_msg_evac(1, 3)

            # layer-1 tail with msg2 interleaved (delay-by-one chunk)
            for ip in range(4):
                rem = list(range(agg1_next_j[ip], NT))
                extras = []
                if ip >= 1:
                    extras = [("mm", c) for c in
                              range(4 * (ip - 1), 4 * ip)] + [("ev", ip - 1)]
                k = max(1, len(rem) // (len(extras) + 1)) if extras else 0
                ei = 0

                def do_extra(e):
                    kind, v = e
                    if kind == "mm":
                        emit_msg_mm(2, v)
                    else:
                        emit_msg_evac(2, v)

                for idx, jj in enumerate(rem):
                    emit_agg_mm(1, ip, jj)
                    if extras and (idx + 1) % k == 0 and ei < len(extras):
                        do_extra(extras[ei])
                        ei += 1
                while ei < len(extras):
                    do_extra(extras[ei])
                    ei += 1
                emit_relu(1, ip)
            for c in range(12, 16):
                emit_msg_mm(2, c)
            emit_msg_evac(2, 3)

            # layer-2 with projection interleaved (delay-by-one chunk)
            for i in range(4):
                extras = list(range(4 * (i - 1), 4 * i)) if i >= 1 else []
                k = max(1, NT // (len(extras) + 1)) if extras else 0
                ei = 0
                for j in range(NT):
                    emit_agg_mm(2, i, j)
                    if extras and (j + 1) % k == 0 and ei < len(extras):
                        c = extras[ei]
                        ei += 1
                        emit_proj_mm(c)
                        if c % 4 == 3:
                            emit_proj_out(c // 8, (c // 4) % 2)
                while ei < len(extras):
                    c = extras[ei]
                    ei += 1
                    emit_proj_mm(c)
                    if c % 4 == 3:
                        emit_proj_out(c // 8, (c // 4) % 2)
                emit_relu(2, i)
            for c in range(12, 16):
                emit_proj_mm(c)
            emit_proj_out(1, 1)

    nc.compile()
    return nc


def kernel(
    latent_features,
    adjacency_matrix,
    node_mask,
    W0,
    b0,
    W1,
    b1,
    W2,
    b2,
    Wout,
    bout,
    _trace=False,
    _agg_dt=None,  # accepted for harness compat; unused
):
    import ml_dtypes

    bf16 = ml_dtypes.bfloat16

    lat = np.asarray(latent_features, dtype=np.float32)
    adj = np.asarray(adjacency_matrix, dtype=np.float32)
    msk = np.asarray(node_mask, dtype=np.float32)
    b0_ = np.asarray(b0, dtype=np.float32).reshape(1, HID)
    b1_ = np.asarray(b1, dtype=np.float32).reshape(1, HID)
    b2_ = np.asarray(b2, dtype=np.float32).reshape(1, HID)
    bo_ = np.asarray(bout, dtype=np.float32).reshape(1, ODIM)
    has_bias = bool(b0_.any() or b1_.any() or b2_.any() or bo_.any())
    has_mask = not bool(np.all(msk == 1.0))

    nc = _build(has_bias, has_mask)

    # A^T bf16 panel-major: ATP[b, i, p, j*512+c] = A[b, i*512+c, j*128+p]
    atp = np.ascontiguousarray(
        adj.transpose(0, 2, 1)
        .reshape(B, NT, 128, NP, 512)
        .transpose(0, 3, 2, 1, 4)
        .astype(bf16)
    ).reshape(B, NP, 128, PANW)
    xt = np.ascontiguousarray(lat.transpose(0, 2, 1)).astype(bf16)
    w0b = np.ascontiguousarray(np.asarray(W0, dtype=np.float32)).astype(bf16)
    w1b = np.ascontiguousarray(np.asarray(W1, dtype=np.float32)).astype(bf16)
    w2b = np.ascontiguousarray(np.asarray(W2, dtype=np.float32)).astype(bf16)
    wob = np.ascontiguousarray(np.asarray(Wout, dtype=np.float32)).astype(bf16)

    in_maps = []
    for c in range(N_CORES):
        m = {
            "ATP": atp[c],
            "XT": xt[c],
            "W0": w0b,
            "W1": w1b,
            "W2": w2b,
            "WO": wob,
        }
        if has_bias:
            m["B0"] = b0_.astype(bf16)
            m["B1"] = b1_.astype(bf16)
            m["B2"] = b2_.astype(bf16)
            m["BO"] = bo_.astype(bf16)
            m["ONE"] = np.ones((1, 128), dtype=bf16)
        if has_mask:
            m["MSK"] = np.ascontiguousarray(msk[c].reshape(NT, 128).T)
        in_maps.append(m)

    res = run_bass_kernel_spmd(
        nc, in_maps, core_ids=list(range(N_CORES)), trace=_trace
    )
    out = np.stack([res.results[c]["Y"] for c in range(N_CORES)], axis=0)
    if _trace:
        return out, res
    return out


# revision 29
# speedup vs baseline: 1.1601x; 1.1601x over previous
"""Trainium2 Bass kernel for a 3-layer dense-adjacency GCN decoder.

Problem (per batch graph): 3x GCN layer (msg = h@W + b; agg = A @ msg; relu)
followed by output projection + node mask. B=8 graphs of N=2048 nodes,
latent=64, hidden=128, out=64. Batch-parallel: one graph per NeuronCore.

Per-core plan (all-bf16 datapath, everything 1 PE-cycle/row):
  - The host hands each core A^T in bf16, panel-major: panel i holds the
    512 target columns i*512..(i+1)*512 for all 2048 source rows.  Each
    panel streams HBM->SBUF as 4 quarter DMAs (fully contiguous rows);
    A^T stays SBUF-resident and is reused by all 3 layers.  bf16 halves
    the HBM traffic of the dominant read (16.8MB -> 8.4MB per core).
  - Features are kept feature-major (h^T: [d, n] bf16; X^T comes
    pre-transposed+cast from the host).  msg is produced NODE-MAJOR
    directly -- msg chunk c = (h^T[:, c*128:+128] stationary) @ W -- so
    no PE transposes anywhere in the pipeline; one PSUM round trip per
    value.
  - Aggregation chunk i accumulates (msg block j stationary) @ (A^T
    panel-i slice, 512-wide moving) over the 16 j-blocks; ReLU rides the
    PSUM->SBUF evacuation (alternating ACT/DVE) straight into the next
    h^T.
  - Schedule: msg0 runs under the stream head; layer-0 agg chunk i is
    paced by panel i's quarter DMAs; msg1 + layer-1 partial aggregation
    steps fill the PE slack under the stream (emitted BEFORE each
    quarter's agg0 group so the in-order PE never head-of-line blocks on
    the DMA).  After the stream the tail is a software pipeline:
    agg1 tail -> relu1 -> msg2 (interleaved) -> agg2 -> relu2 -> proj
    (delay-by-one) -> masked evac -> Y DMA halves.
  - Bias / node-mask are supported via build flags resolved on the host
    at call time (bias = one extra rank-1 accumulating matmul per msg
    chunk; mask = per-partition ACT scale on the output evacuation).
    With zero biases / unit mask those instructions are not emitted.
"""

import functools

import numpy as np

import concourse.bass as bass
import concourse.bacc as bacc
import concourse.tile as tile
from concourse import mybir
from concourse.bass_utils import run_bass_kernel_spmd

B = 8
N = 2048
NT = N // 128  # 16 node blocks
NP = 4         # target panels (512 columns each)
NQ = 4         # quarter DMAs per panel
LAT = 64
HID = 128
ODIM = 64
N_CORES = 8

PANW = NT * 512  # at_t columns per panel
QW = PANW // NQ  # at_t columns per quarter DMA

F32 = mybir.dt.float32
BF16 = mybir.dt.bfloat16
Act = mybir.ActivationFunctionType


@functools.lru_cache(maxsize=4)
def _build(has_bias: bool, has_mask: bool):
    nc = bacc.Bacc(None, target_bir_lowering=False, debug=False)

    ATP_d = nc.declare_dram_parameter("ATP", [NP, 128, PANW], BF16,
                                      isOutput=False)
    XT_d = nc.declare_dram_parameter("XT", [LAT, N], BF16, isOutput=False)
    W0_d = nc.declare_dram_parameter("W0", [LAT, HID], BF16, isOutput=False)
    W1_d = nc.declare_dram_parameter("W1", [HID, HID], BF16, isOutput=False)
    W2_d = nc.declare_dram_parameter("W2", [HID, HID], BF16, isOutput=False)
    WO_d = nc.declare_dram_parameter("WO", [HID, ODIM], BF16, isOutput=False)
    if has_bias:
        B0_d = nc.declare_dram_parameter("B0", [1, HID], BF16, isOutput=False)
        B1_d = nc.declare_dram_parameter("B1", [1, HID], BF16, isOutput=False)
        B2_d = nc.declare_dram_parameter("B2", [1, HID], BF16, isOutput=False)
        BO_d = nc.declare_dram_parameter("BO", [1, ODIM], BF16, isOutput=False)
        ONE_d = nc.declare_dram_parameter("ONE", [1, 128], BF16,
                                          isOutput=False)
    if has_mask:
        MSK_d = nc.declare_dram_parameter("MSK", [128, NT], F32,
                                          isOutput=False)
    Y_d = nc.declare_dram_parameter("Y", [N, ODIM], F32, isOutput=True)
    Y3 = Y_d[:].rearrange("(t p) f -> p t f", p=128)

    with tile.TileContext(nc) as tc:
        with (
            tc.tile_pool(name="const", bufs=1) as constp,
            tc.tile_pool(name="at", bufs=1) as atp,
            tc.tile_pool(name="x", bufs=1) as xp,
            tc.tile_pool(name="ht", bufs=3) as htp,
            tc.tile_pool(name="msg", bufs=3) as msgp,
            tc.tile_pool(name="out", bufs=1) as outp,
            tc.tile_pool(name="agg", bufs=4, space=bass.MemorySpace.PSUM)
            as aggp,
            tc.tile_pool(name="mp", bufs=3, space=bass.MemorySpace.PSUM)
            as mpp,
        ):
            at_t = atp.tile([128, NP * PANW], BF16, tag="at")

            # ---- sync queue: W0 + X^T (halved) ride ahead of the A^T
            # stream (the wire serves this queue in order; anything on the
            # other queue starves behind the 8.4MB stream) ----
            w0 = constp.tile([LAT, HID], BF16, tag="w0")
            nc.sync.dma_start(w0[:], W0_d[:])
            hT0 = xp.tile([LAT, N], BF16, tag="x")
            nc.sync.dma_start(hT0[:], XT_d[:])
            for i in range(NP):
                # panel 0 streams in eighths so agg0 starts on the first
                # piece; later panels use quarters (fewer issue slots)
                npieces = 8 if i == 0 else NQ
                pw = PANW // npieces
                for q in range(npieces):
                    c0 = i * PANW + q * pw
                    nc.sync.dma_start(
                        at_t[:, c0 : c0 + pw],
                        ATP_d[i, :, q * pw : (q + 1) * pw],
                    )

            # ---- gpsimd queue: the later constants ----
            w1 = constp.tile([HID, HID], BF16, tag="w1")
            nc.gpsimd.dma_start(w1[:], W1_d[:])
            w2 = constp.tile([HID, HID], BF16, tag="w2")
            nc.gpsimd.dma_start(w2[:], W2_d[:])
            wo = constp.tile([HID, ODIM], BF16, tag="wo")
            nc.gpsimd.dma_start(wo[:], WO_d[:])
            if has_bias:
                one_t = constp.tile([1, 128], BF16, tag="one")
                nc.gpsimd.dma_start(one_t[:], ONE_d[:])
                bcols = []
                for name, d_, bd in (("b0", HID, B0_d), ("b1", HID, B1_d),
                                     ("b2", HID, B2_d), ("bo", ODIM, BO_d)):
                    bt = constp.tile([1, d_], BF16, tag=name)
                    nc.gpsimd.dma_start(bt[:], bd[:])
                    bcols.append(bt)
            if has_mask:
                mskn = constp.tile([128, NT], F32, tag="mskn")
                nc.gpsimd.dma_start(mskn[:], MSK_d[:])

            hts = [hT0]
            for l in range(3):
                hts.append(htp.tile([HID, N], BF16, tag="ht",
                                    name=f"hT{l + 1}"))
            msgs = [msgp.tile([128, N], BF16, tag="msg", name=f"msg{l}")
                    for l in range(3)]
            out_sb = outp.tile([128, NT * ODIM], F32, tag="out")

            ws = [w0, w1, w2]
            din = [LAT, HID, HID]

            tog = [0]

            def alt():
                tog[0] ^= 1
                return tog[0]

            # ---- msg stage: node-major msg chunks, 4 per PSUM bank ----
            mp_tiles = {}

            def emit_msg_mm(l, c):
                g, q = divmod(c, 4)
                if q == 0:
                    mp_tiles[(l, g)] = mpp.tile([128, 512], F32, tag="mp",
                                                name=f"mp{l}_{g}")
                mp = mp_tiles[(l, g)]
                dst = mp[:, q * HID : (q + 1) * HID]
                nc.tensor.matmul(
                    dst,
                    hts[l][0 : din[l], c * 128 : (c + 1) * 128],
                    ws[l][:],
                    start=True,
                    stop=not has_bias,
                )
                if has_bias:
                    nc.tensor.matmul(
                        dst, one_t[:], bcols[l][:], start=False, stop=True
                    )

            def emit_msg_evac(l, g):
                # halves on both engines concurrently: dependents unblock
                # in ~half the evacuation latency
                mp = mp_tiles.pop((l, g))
                dst = msgs[l][:, g * 512 : (g + 1) * 512]
                nc.scalar.activation(dst[:, 0:256], mp[:, 0:256], Act.Identity)
                nc.vector.tensor_copy(dst[:, 256:512], mp[:, 256:512])

            # ---- aggregation: 16 accumulating j-steps per 512-col chunk ----
            agg_tiles = {}
            agg_cnt = {}

            def emit_agg_mm(l, i, j):
                key = (l, i)
                if key not in agg_tiles:
                    agg_tiles[key] = aggp.tile([128, 512], F32, tag="agg",
                                               name=f"agg{l}_{i}")
                    agg_cnt[key] = 0
                n = agg_cnt[key]
                nc.tensor.matmul(
                    agg_tiles[key][:],
                    msgs[l][:, j * 128 : (j + 1) * 128],
                    at_t[:, i * PANW + j * 512 : i * PANW + (j + 1) * 512],
                    start=(n == 0),
                    stop=(n == NT - 1),
                )
                agg_cnt[key] = n + 1

            def emit_relu(l, i):
                ap_ps = agg_tiles.pop((l, i))
                dst = hts[l + 1][:, i * 512 : (i + 1) * 512]
                nc.scalar.activation(dst[:, 0:256], ap_ps[:, 0:256], Act.Relu)
                nc.vector.tensor_scalar_max(
                    dst[:, 256:512], ap_ps[:, 256:512], 0.0
                )

            # ---- projection: node-major out chunks, 8 per PSUM bank ----
            pp_tiles = {}

            def emit_proj_mm(c):
                b, s = divmod(c, 8)
                if s == 0:
                    pp_tiles[b] = mpp.tile([128, 512], F32, tag="mp",
                                           name=f"pp{b}")
                dst = pp_tiles[b][:, s * ODIM : (s + 1) * ODIM]
                nc.tensor.matmul(
                    dst,
                    hts[3][:, c * 128 : (c + 1) * 128],
                    wo[:],
                    start=True,
                    stop=not has_bias,
                )
                if has_bias:
                    nc.tensor.matmul(
                        dst, one_t[:], bcols[3][:], start=False, stop=True
                    )

            def emit_proj_out(bk, half):
                # evacuate + DMA one half-bank (4 node blocks, 128KB)
                pp = pp_tiles[bk] if half == 0 else pp_tiles.pop(bk)
                if has_mask:
                    for s in range(4 * half, 4 * half + 4):
                        c = bk * 8 + s
                        col = mskn[:, c : c + 1]
                        dst = out_sb[:, c * ODIM : (c + 1) * ODIM]
                        src = pp[:, s * ODIM : (s + 1) * ODIM]
                        if alt():
                            nc.scalar.activation(dst, src, Act.Copy,
                                                 scale=col)
                        else:
                            nc.vector.tensor_scalar_mul(dst, src, col)
                else:
                    o = bk * 512 + half * 256
                    dst = out_sb[:, o : o + 256]
                    src = pp[:, half * 256 : half * 256 + 256]
                    nc.scalar.activation(dst[:, 0:128], src[:, 0:128],
                                         Act.Identity)
                    nc.vector.tensor_copy(dst[:, 128:256], src[:, 128:256])
                t0 = bk * 8 + half * 4
                nc.sync.dma_start(
                    Y3[:, t0 : t0 + 4, :],
                    out_sb[:].rearrange("p (t f) -> p t f", f=ODIM)[
                        :, t0 : t0 + 4, :
                    ],
                )

            # ================= schedule =================
            # msg0 for all 16 chunks (depends only on X^T + W0)
            for c in range(NT):
                emit_msg_mm(0, c)
                if c % 4 == 3:
                    emit_msg_evac(0, c // 4)

            # layer-0 agg paced by the stream; msg1 + agg1 partials fill
            # the slack.  Ready work is emitted BEFORE each quarter's
            # agg0 group so the in-order PE never blocks on the DMA.
            agg1_next_j = [0, 0, 0, 0]

            def agg1_avail(jmax, ipmax):
                steps = []
                for ip in range(ipmax + 1):
                    while agg1_next_j[ip] < jmax:
                        steps.append((ip, agg1_next_j[ip]))
                        agg1_next_j[ip] += 1
                return steps

            for i in range(NP):
                for q in range(NQ):
                    if i >= 1 and q == 0:
                        # all msg1 work for the finished panel up front --
                        # the LS stall lands inside the panel-boundary
                        # DMA wait
                        for c in range(4 * (i - 1), 4 * i):
                            emit_msg_mm(1, c)
                        emit_msg_evac(1, i - 1)
                    if i >= 1 and q >= 1:
                        # agg1 partials fill the slack under the stream
                        for ip, jj in agg1_avail(4 * i, i - 1):
                            emit_agg_mm(1, ip, jj)
                    for j in range(4 * q, 4 * q + 4):
                        emit_agg_mm(0, i, j)
                emit_relu(0, i)

            # stream done: msg1 tail
            for c in range(12, 16):
                emit_msg_mm(1, c)
            emit_msg_evac(1, 3)

            # layer-1 tail with msg2 interleaved (delay-by-one chunk)
            for ip in range(4):
                rem = list(range(agg1_next_j[ip], NT))
                extras = []
                if ip >= 1:
                    extras = [("mm", c) for c in
                              range(4 * (ip - 1), 4 * ip)] + [("ev", ip - 1)]
                k = max(1, len(rem) // (len(extras) + 1)) if extras else 0
                ei = 0

                def do_extra(e):
                    kind, v = e
                    if kind == "mm":
                        emit_msg_mm(2, v)
                    else:
                        emit_msg_evac(2, v)

                for idx, jj in enumerate(rem):
                    emit_agg_mm(1, ip, jj)
                    if extras and (idx + 1) % k == 0 and ei < len(extras):
                        do_extra(extras[ei])
                        ei += 1
                while ei < len(extras):
                    do_extra(extras[ei])
                    ei += 1
                emit_relu(1, ip)
            for c in range(12, 16):
                emit_msg_mm(2, c)
            emit_msg_evac(2, 3)

            # layer-2 with projection interleaved (delay-by-one chunk)
            for i in range(4):
                extras = list(range(4 * (i - 1), 4 * i)) if i >= 1 else []
                k = max(1, NT // (len(extras) + 1)) if extras else 0
                ei = 0
                for j in range(NT):
                    emit_agg_mm(2, i, j)
                    if extras and (j + 1) % k == 0 and ei < len(extras):
                        c = extras[ei]
                        ei += 1
                        emit_proj_mm(c)
                        if c % 4 == 3:
                            emit_proj_out(c // 8, (c // 4) % 2)
                while ei < len(extras):
                    c = extras[ei]
                    ei += 1
                    emit_proj_mm(c)
                    if c % 4 == 3:
                        emit_proj_out(c // 8, (c // 4) % 2)
                emit_relu(2, i)
            for c in range(12, 16):
                emit_proj_mm(c)
            emit_proj_out(1, 1)

    nc.compile()
    return nc


def kernel(
    latent_features,
    adjacency_matrix,
    node_mask,
    W0,
    b0,
    W1,
    b1,
    W2,
    b2,
    Wout,
    bout,
    _trace=False,
    _agg_dt=None,  # accepted for harness compat; unused
):
    import ml_dtypes

    bf16 = ml_dtypes.bfloat16

    lat = np.asarray(latent_features, dtype=np.float32)
    adj = np.asarray(adjacency_matrix, dtype=np.float32)
    msk = np.asarray(node_mask, dtype=np.float32)
    b0_ = np.asarray(b0, dtype=np.float32).reshape(1, HID)
    b1_ = np.asarray(b1, dtype=np.float32).reshape(1, HID)
    b2_ = np.asarray(b2, dtype=np.float32).reshape(1, HID)
    bo_ = np.asarray(bout, dtype=np.float32).reshape(1, ODIM)
    has_bias = bool(b0_.any() or b1_.any() or b2_.any() or bo_.any())
    has_mask = not bool(np.all(msk == 1.0))

    nc = _build(has_bias, has_mask)

    # A^T bf16 panel-major: ATP[b, i, p, j*512+c] = A[b, i*512+c, j*128+p]
    atp = np.ascontiguousarray(
        adj.transpose(0, 2, 1)
        .reshape(B, NT, 128, NP, 512)
        .transpose(0, 3, 2, 1, 4)
        .astype(bf16)
    ).reshape(B, NP, 128, PANW)
    xt = np.ascontiguousarray(lat.transpose(0, 2, 1)).astype(bf16)
    w0b = np.ascontiguousarray(np.asarray(W0, dtype=np.float32)).astype(bf16)
    w1b = np.ascontiguousarray(np.asarray(W1, dtype=np.float32)).astype(bf16)
    w2b = np.ascontiguousarray(np.asarray(W2, dtype=np.float32)).astype(bf16)
    wob = np.ascontiguousarray(np.asarray(Wout, dtype=np.float32)).astype(bf16)

    in_maps = []
    for c in range(N_CORES):
        m = {
            "ATP": atp[c],
            "XT": xt[c],
            "W0": w0b,
            "W1": w1b,
            "W2": w2b,
            "WO": wob,
        }
        if has_bias:
            m["B0"] = b0_.astype(bf16)
            m["B1"] = b1_.astype(bf16)
            m["B2"] = b2_.astype(bf16)
            m["BO"] = bo_.astype(bf16)
            m["ONE"] = np.ones((1, 128), dtype=bf16)
        if has_mask:
            m["MSK"] = np.ascontiguousarray(msk[c].reshape(NT, 128).T)
        in_maps.append(m)

    res = run_bass_kernel_spmd(
        nc, in_maps, core_ids=list(range(N_CORES)), trace=_trace
    )
    out = np.stack([res.results[c]["Y"] for c in range(N_CORES)], axis=0)
    if _trace:
        return out, res
    return out


# revision 30
# speedup vs baseline: 1.1742x; 1.0121x over previous
"""Trainium2 Bass kernel for a 3-layer dense-adjacency GCN decoder.

Problem (per batch graph): 3x GCN layer (msg = h@W + b; agg = A @ msg; relu)
followed by output projection + node mask. B=8 graphs of N=2048 nodes,
latent=64, hidden=128, out=64. Batch-parallel: one graph per NeuronCore.

Per-core plan (all-bf16 datapath, everything 1 PE-cycle/row):
  - The host hands each core A^T in bf16, panel-major: panel i holds the
    512 target columns i*512..(i+1)*512 for all 2048 source rows.  Each
    panel streams HBM->SBUF as 4 quarter DMAs (fully contiguous rows);
    A^T stays SBUF-resident and is reused by all 3 layers.  bf16 halves
    the HBM traffic of the dominant read (16.8MB -> 8.4MB per core).
  - Features are kept feature-major (h^T: [d, n] bf16; X^T comes
    pre-transposed+cast from the host).  msg is produced NODE-MAJOR
    directly -- msg chunk c = (h^T[:, c*128:+128] stationary) @ W -- so
    no PE transposes anywhere in the pipeline; one PSUM round trip per
    value.
  - Aggregation chunk i accumulates (msg block j stationary) @ (A^T
    panel-i slice, 512-wide moving) over the 16 j-blocks; ReLU rides the
    PSUM->SBUF evacuation (alternating ACT/DVE) straight into the next
    h^T.
  - Schedule: msg0 runs under the stream head; layer-0 agg chunk i is
    paced by panel i's quarter DMAs; msg1 + layer-1 partial aggregation
    steps fill the PE slack under the stream (emitted BEFORE each
    quarter's agg0 group so the in-order PE never head-of-line blocks on
    the DMA).  After the stream the tail is a software pipeline:
    agg1 tail -> relu1 -> msg2 (interleaved) -> agg2 -> relu2 -> proj
    (delay-by-one) -> masked evac -> Y DMA halves.
  - Bias / node-mask are supported via build flags resolved on the host
    at call time (bias = one extra rank-1 accumulating matmul per msg
    chunk; mask = per-partition ACT scale on the output evacuation).
    With zero biases / unit mask those instructions are not emitted.
"""

import functools

import numpy as np

import concourse.bass as bass
import concourse.bacc as bacc
import concourse.tile as tile
from concourse import mybir
from concourse.bass_utils import run_bass_kernel_spmd

B = 8
N = 2048
NT = N // 128  # 16 node blocks
NP = 4         # target panels (512 columns each)
NQ = 4         # quarter DMAs per panel
LAT = 64
HID = 128
ODIM = 64
N_CORES = 8

PANW = NT * 512  # at_t columns per panel
QW = PANW // NQ  # at_t columns per quarter DMA

F32 = mybir.dt.float32
BF16 = mybir.dt.bfloat16
Act = mybir.ActivationFunctionType


@functools.lru_cache(maxsize=4)
def _build(has_bias: bool, has_mask: bool):
    nc = bacc.Bacc(None, target_bir_lowering=False, debug=False)

    ATP_d = nc.declare_dram_parameter("ATP", [NP, 128, PANW], BF16,
                                      isOutput=False)
    XT_d = nc.declare_dram_parameter("XT", [LAT, N], BF16, isOutput=False)
    W0_d = nc.declare_dram_parameter("W0", [LAT, HID], BF16, isOutput=False)
    W1_d = nc.declare_dram_parameter("W1", [HID, HID], BF16, isOutput=False)
    W2_d = nc.declare_dram_parameter("W2", [HID, HID], BF16, isOutput=False)
    WO_d = nc.declare_dram_parameter("WO", [HID, ODIM], BF16, isOutput=False)
    if has_bias:
        B0_d = nc.declare_dram_parameter("B0", [1, HID], BF16, isOutput=False)
        B1_d = nc.declare_dram_parameter("B1", [1, HID], BF16, isOutput=False)
        B2_d = nc.declare_dram_parameter("B2", [1, HID], BF16, isOutput=False)
        BO_d = nc.declare_dram_parameter("BO", [1, ODIM], BF16, isOutput=False)
        ONE_d = nc.declare_dram_parameter("ONE", [1, 128], BF16,
                                          isOutput=False)
    if has_mask:
        MSK_d = nc.declare_dram_parameter("MSK", [128, NT], F32,
                                          isOutput=False)
    Y_d = nc.declare_dram_parameter("Y", [N, ODIM], F32, isOutput=True)
    Y3 = Y_d[:].rearrange("(t p) f -> p t f", p=128)

    with tile.TileContext(nc) as tc:
        with (
            tc.tile_pool(name="const", bufs=1) as constp,
            tc.tile_pool(name="at", bufs=1) as atp,
            tc.tile_pool(name="x", bufs=1) as xp,
            tc.tile_pool(name="ht", bufs=3) as htp,
            tc.tile_pool(name="msg", bufs=3) as msgp,
            tc.tile_pool(name="out", bufs=1) as outp,
            tc.tile_pool(name="agg", bufs=4, space=bass.MemorySpace.PSUM)
            as aggp,
            tc.tile_pool(name="mp", bufs=2, space=bass.MemorySpace.PSUM)
            as mpp,
        ):
            at_t = atp.tile([128, NP * PANW], BF16, tag="at")

            # ---- sync queue: W0 + X^T (halved) ride ahead of the A^T
            # stream (the wire serves this queue in order; anything on the
            # other queue starves behind the 8.4MB stream) ----
            w0 = constp.tile([LAT, HID], BF16, tag="w0")
            nc.sync.dma_start(w0[:], W0_d[:])
            hT0 = xp.tile([LAT, N], BF16, tag="x")
            nc.sync.dma_start(hT0[:], XT_d[:])
            for i in range(NP):
                # panel 0 streams in eighths so agg0 starts on the first
                # piece; later panels use quarters (fewer issue slots)
                npieces = 8 if i == 0 else NQ
                pw = PANW // npieces
                for q in range(npieces):
                    c0 = i * PANW + q * pw
                    nc.sync.dma_start(
                        at_t[:, c0 : c0 + pw],
                        ATP_d[i, :, q * pw : (q + 1) * pw],
                    )

            # ---- gpsimd queue: the later constants ----
            w1 = constp.tile([HID, HID], BF16, tag="w1")
            nc.gpsimd.dma_start(w1[:], W1_d[:])
            w2 = constp.tile([HID, HID], BF16, tag="w2")
            nc.gpsimd.dma_start(w2[:], W2_d[:])
            wo = constp.tile([HID, ODIM], BF16, tag="wo")
            nc.gpsimd.dma_start(wo[:], WO_d[:])
            if has_bias:
                one_t = constp.tile([1, 128], BF16, tag="one")
                nc.gpsimd.dma_start(one_t[:], ONE_d[:])
                bcols = []
                for name, d_, bd in (("b0", HID, B0_d), ("b1", HID, B1_d),
                                     ("b2", HID, B2_d), ("bo", ODIM, BO_d)):
                    bt = constp.tile([1, d_], BF16, tag=name)
                    nc.gpsimd.dma_start(bt[:], bd[:])
                    bcols.append(bt)
            if has_mask:
                mskn = constp.tile([128, NT], F32, tag="mskn")
                nc.gpsimd.dma_start(mskn[:], MSK_d[:])

            hts = [hT0]
            for l in range(3):
                hts.append(htp.tile([HID, N], BF16, tag="ht",
                                    name=f"hT{l + 1}"))
            msgs = [msgp.tile([128, N], BF16, tag="msg", name=f"msg{l}")
                    for l in range(3)]
            out_sb = outp.tile([128, NT * ODIM], F32, tag="out")

            ws = [w0, w1, w2]
            din = [LAT, HID, HID]

            tog = [0]

            def alt():
                tog[0] ^= 1
                return tog[0]

            # ---- msg stage: node-major msg chunks, 4 per PSUM bank ----
            mp_tiles = {}

            def emit_msg_mm(l, c):
                g, q = divmod(c, 4)
                if q == 0:
                    mp_tiles[(l, g)] = mpp.tile([128, 512], F32, tag="mp",
                                                name=f"mp{l}_{g}")
                mp = mp_tiles[(l, g)]
                dst = mp[:, q * HID : (q + 1) * HID]
                nc.tensor.matmul(
                    dst,
                    hts[l][0 : din[l], c * 128 : (c + 1) * 128],
                    ws[l][:],
                    start=True,
                    stop=not has_bias,
                )
                if has_bias:
                    nc.tensor.matmul(
                        dst, one_t[:], bcols[l][:], start=False, stop=True
                    )

            def emit_msg_evac(l, g):
                # halves on both engines concurrently: dependents unblock
                # in ~half the evacuation latency
                mp = mp_tiles.pop((l, g))
                dst = msgs[l][:, g * 512 : (g + 1) * 512]
                nc.scalar.activation(dst[:, 0:256], mp[:, 0:256], Act.Identity)
                nc.vector.tensor_copy(dst[:, 256:512], mp[:, 256:512])

            # ---- aggregation: 16 accumulating j-steps per 512-col chunk ----
            agg_tiles = {}
            agg_cnt = {}

            def emit_agg_mm(l, i, j):
                key = (l, i)
                if key not in agg_tiles:
                    agg_tiles[key] = aggp.tile([128, 512], F32, tag="agg",
                                               name=f"agg{l}_{i}")
                    agg_cnt[key] = 0
                n = agg_cnt[key]
                nc.tensor.matmul(
                    agg_tiles[key][:],
                    msgs[l][:, j * 128 : (j + 1) * 128],
                    at_t[:, i * PANW + j * 512 : i * PANW + (j + 1) * 512],
                    start=(n == 0),
                    stop=(n == NT - 1),
                )
                agg_cnt[key] = n + 1

            def emit_relu(l, i):
                ap_ps = agg_tiles.pop((l, i))
                dst = hts[l + 1][:, i * 512 : (i + 1) * 512]
                nc.scalar.activation(dst[:, 0:256], ap_ps[:, 0:256], Act.Relu)
                nc.vector.tensor_scalar_max(
                    dst[:, 256:512], ap_ps[:, 256:512], 0.0
                )

            # ---- projection: node-major out chunks, 8 per PSUM bank ----
            pp_tiles = {}

            def emit_proj_mm(c):
                b, s = divmod(c, 8)
                if s == 0:
                    pp_tiles[b] = mpp.tile([128, 512], F32, tag="mp",
                                           name=f"pp{b}")
                dst = pp_tiles[b][:, s * ODIM : (s + 1) * ODIM]
                nc.tensor.matmul(
                    dst,
                    hts[3][:, c * 128 : (c + 1) * 128],
                    wo[:],
                    start=True,
                    stop=not has_bias,
                )
                if has_bias:
                    nc.tensor.matmul(
                        dst, one_t[:], bcols[3][:], start=False, stop=True
                    )

            def emit_proj_out(bk, half):
                # evacuate + DMA one half-bank (4 node blocks, 128KB)
                pp = pp_tiles[bk] if half == 0 else pp_tiles.pop(bk)
                if has_mask:
                    for s in range(4 * half, 4 * half + 4):
                        c = bk * 8 + s
                        col = mskn[:, c : c + 1]
                        dst = out_sb[:, c * ODIM : (c + 1) * ODIM]
                        src = pp[:, s * ODIM : (s + 1) * ODIM]
                        if alt():
                            nc.scalar.activation(dst, src, Act.Copy,
                                                 scale=col)
                        else:
                            nc.vector.tensor_scalar_mul(dst, src, col)
                else:
                    o = bk * 512 + half * 256
                    dst = out_sb[:, o : o + 256]
                    src = pp[:, half * 256 : half * 256 + 256]
                    nc.scalar.activation(dst[:, 0:128], src[:, 0:128],
                                         Act.Identity)
                    nc.vector.tensor_copy(dst[:, 128:256], src[:, 128:256])
                t0 = bk * 8 + half * 4
                nc.sync.dma_start(
                    Y3[:, t0 : t0 + 4, :],
                    out_sb[:].rearrange("p (t f) -> p t f", f=ODIM)[
                        :, t0 : t0 + 4, :
                    ],
                )

            # ================= schedule =================
            # msg0 for all 16 chunks (depends only on X^T + W0)
            for c in range(NT):
                emit_msg_mm(0, c)
                if c % 4 == 3:
                    emit_msg_evac(0, c // 4)

            # layer-0 agg paced by the stream; msg1 + agg1 partials fill
            # the slack.  Ready work is emitted BEFORE each quarter's
            # agg0 group so the in-order PE never blocks on the DMA.
            agg1_next_j = [0, 0, 0, 0]

            def agg1_avail(jmax, ipmax):
                steps = []
                for ip in range(ipmax + 1):
                    while agg1_next_j[ip] < jmax:
                        steps.append((ip, agg1_next_j[ip]))
                        agg1_next_j[ip] += 1
                return steps

            for i in range(NP):
                for q in range(NQ):
                    if i >= 1 and q == 0:
                        # all msg1 work for the finished panel up front --
                        # the LS stall lands inside the panel-boundary
                        # DMA wait
                        for c in range(4 * (i - 1), 4 * i):
                            emit_msg_mm(1, c)
                        emit_msg_evac(1, i - 1)
                    if i >= 1 and q >= 1:
                        # agg1 partials fill the slack under the stream
                        for ip, jj in agg1_avail(4 * i, i - 1):
                            emit_agg_mm(1, ip, jj)
                    for j in range(4 * q, 4 * q + 4):
                        emit_agg_mm(0, i, j)
                emit_relu(0, i)

            # stream done: msg1 tail
            for c in range(12, 16):
                emit_msg_mm(1, c)
            emit_msg_evac(1, 3)

            # layer-1 tail with msg2 interleaved (delay-by-one chunk)
            for ip in range(4):
                rem = list(range(agg1_next_j[ip], NT))
                extras = []
                if ip >= 1:
                    extras = [("mm", c) for c in
                              range(4 * (ip - 1), 4 * ip)] + [("ev", ip - 1)]
                k = max(1, len(rem) // (len(extras) + 1)) if extras else 0
                ei = 0

                def do_extra(e):
                    kind, v = e
                    if kind == "mm":
                        emit_msg_mm(2, v)
                    else:
                        emit_msg_evac(2, v)

                for idx, jj in enumerate(rem):
                    emit_agg_mm(1, ip, jj)
                    if extras and (idx + 1) % k == 0 and ei < len(extras):
                        do_extra(extras[ei])
                        ei += 1
                while ei < len(extras):
                    do_extra(extras[ei])
                    ei += 1
                emit_relu(1, ip)
            for c in range(12, 16):
                emit_msg_mm(2, c)
            emit_msg_evac(2, 3)

            # layer-2 with projection interleaved (delay-by-one chunk)
            for i in range(4):
                extras = list(range(4 * (i - 1), 4 * i)) if i >= 1 else []
                k = max(1, NT // (len(extras) + 1)) if extras else 0
                ei = 0
                for j in range(NT):
                    emit_agg_mm(2, i, j)
                    if extras and (j + 1) % k == 0 and ei < len(extras):
                        c = extras[ei]
                        ei += 1
                        emit_proj_mm(c)
                        if c % 4 == 3:
                            emit_proj_out(c // 8, (c // 4) % 2)
                while ei < len(extras):
                    c = extras[ei]
                    ei += 1
                    emit_proj_mm(c)
                    if c % 4 == 3:
                        emit_proj_out(c // 8, (c // 4) % 2)
                emit_relu(2, i)
            for c in range(12, 16):
                emit_proj_mm(c)
            emit_proj_out(1, 1)

    nc.compile()
    return nc


def kernel(
    latent_features,
    adjacency_matrix,
    node_mask,
    W0,
    b0,
    W1,
    b1,
    W2,
    b2,
    Wout,
    bout,
    _trace=False,
    _agg_dt=None,  # accepted for harness compat; unused
):
    import ml_dtypes

    bf16 = ml_dtypes.bfloat16

    lat = np.asarray(latent_features, dtype=np.float32)
    adj = np.asarray(adjacency_matrix, dtype=np.float32)
    msk = np.asarray(node_mask, dtype=np.float32)
    b0_ = np.asarray(b0, dtype=np.float32).reshape(1, HID)
    b1_ = np.asarray(b1, dtype=np.float32).reshape(1, HID)
    b2_ = np.asarray(b2, dtype=np.float32).reshape(1, HID)
    bo_ = np.asarray(bout, dtype=np.float32).reshape(1, ODIM)
    has_bias = bool(b0_.any() or b1_.any() or b2_.any() or bo_.any())
    has_mask = not bool(np.all(msk == 1.0))

    nc = _build(has_bias, has_mask)

    # A^T bf16 panel-major: ATP[b, i, p, j*512+c] = A[b, i*512+c, j*128+p]
    atp = np.ascontiguousarray(
        adj.transpose(0, 2, 1)
        .reshape(B, NT, 128, NP, 512)
        .transpose(0, 3, 2, 1, 4)
        .astype(bf16)
    ).reshape(B, NP, 128, PANW)
    xt = np.ascontiguousarray(lat.transpose(0, 2, 1)).astype(bf16)
    w0b = np.ascontiguousarray(np.asarray(W0, dtype=np.float32)).astype(bf16)
    w1b = np.ascontiguousarray(np.asarray(W1, dtype=np.float32)).astype(bf16)
    w2b = np.ascontiguousarray(np.asarray(W2, dtype=np.float32)).astype(bf16)
    wob = np.ascontiguousarray(np.asarray(Wout, dtype=np.float32)).astype(bf16)

    in_maps = []
    for c in range(N_CORES):
        m = {
            "ATP": atp[c],
            "XT": xt[c],
            "W0": w0b,
            "W1": w1b,
            "W2": w2b,
            "WO": wob,
        }
        if has_bias:
            m["B0"] = b0_.astype(bf16)
            m["B1"] = b1_.astype(bf16)
            m["B2"] = b2_.astype(bf16)
            m["BO"] = bo_.astype(bf16)
            m["ONE"] = np.ones((1, 128), dtype=bf16)
        if has_mask:
            m["MSK"] = np.ascontiguousarray(msk[c].reshape(NT, 128).T)
        in_maps.append(m)

    res = run_bass_kernel_spmd(
        nc, in_maps, core_ids=list(range(N_CORES)), trace=_trace
    )
    out = np.stack([res.results[c]["Y"] for c in range(N_CORES)], axis=0)
    if _trace:
        return out, res
    return out
